# revision 2
# baseline (speedup 1.0000x reference)
"""Causal self-attention (12 heads, T=1024, C=768, prefix P=4) on 8 TRN2 cores.

Sharding: data-parallel over batch B=8 -> one batch element per NeuronCore.
No collectives. Weights are replicated to every core.

Per-core kernel (all fp32):
  qkv projection split by destination layout:
    qT,kT  [128, T] per head-pair (transposed layout) = w_attn_slice.T @ xT
    v      [T, 12*65] natural layout (65th col per head = 1.0 for the
           softmax denominator), = xT_slice.T @ w_v
  prefix k/v (4 positions) are appended at the END of the kv axis, so the
  causal structure is block lower-triangular in (kv-chunk, t-chunk) space:
    scores^T tile (r, window c): psum = kT_slice.T @ qT_window  [128kv, 512t]
    e = exp(0.125 * psum)  (no max subtraction: |scores| ~ 2)
    diagonal band tiles multiplied by a 128x128 triangular 0/1 mask;
    fully-masked columns are never computed nor read.
  AV: y[tchunk] = sum_r e^T(r).T @ v_aug(r)  -> psum [128t, 65]
    col 64 = softmax denominator; normalize via DVE reciprocal +
    per-partition tensor_scalar_mul.  Two heads share a [128,128] y tile,
    one PE transpose each -> yT pair tiles [128, T].
  out = yT.T @ w_proj + b_proj  -> [T, 768] -> DMA out.
"""

import numpy as np
from contextlib import ExitStack

import concourse.bass as bass
import concourse.mybir as mybir
import concourse.tile as tile
from concourse import bacc
from concourse.bass_utils import run_bass_kernel_spmd

F32 = mybir.dt.float32
N_CORES = 8
T, C, H, D, PFX = 1024, 768, 12, 64, 4
NPAIR = H // 2          # 6 head pairs
KC = C // 128           # 6 contraction chunks
W = 512                 # T window for scores
NW = T // W             # 2 windows
TCH = T // 128          # 8 T chunks
EXP = mybir.ActivationFunctionType.Exp
SCALE = 1.0 / np.sqrt(D)


def _build():
    nc = bacc.Bacc("TRN2", target_bir_lowering=False, debug=False,
                   num_devices=N_CORES)
    xT_d = nc.declare_dram_parameter("xT", [C, T], F32, isOutput=False)
    wqk_d = nc.declare_dram_parameter("w_qk", [C, 2 * C], F32, isOutput=False)
    wv_d = nc.declare_dram_parameter("w_v", [C, C], F32, isOutput=False)
    wp_d = nc.declare_dram_parameter("w_p", [C, C], F32, isOutput=False)
    bqk_d = nc.declare_dram_parameter("b_qk", [128, 12], F32, isOutput=False)
    bv_d = nc.declare_dram_parameter("bv_bc", [128, C], F32, isOutput=False)
    bp_d = nc.declare_dram_parameter("bp_bc", [128, C], F32, isOutput=False)
    kTc_d = nc.declare_dram_parameter("kTc", [C, PFX], F32, isOutput=False)
    vc_d = nc.declare_dram_parameter("vc_aug", [PFX, H, 65], F32, isOutput=False)
    tri_d = nc.declare_dram_parameter("tri", [128, 128], F32, isOutput=False)
    id_d = nc.declare_dram_parameter("ident", [128, 128], F32, isOutput=False)
    out_d = nc.declare_dram_parameter("out", [T, C], F32, isOutput=True)

    with tile.TileContext(nc) as tc, ExitStack() as ctx:
        pers = ctx.enter_context(tc.tile_pool(name="pers", bufs=1))
        wqkp = ctx.enter_context(tc.tile_pool(name="wqkp", bufs=12))
        qkp = ctx.enter_context(tc.tile_pool(name="qkp", bufs=2))
        ep = ctx.enter_context(tc.tile_pool(name="ep", bufs=14))
        epp = ctx.enter_context(tc.tile_pool(name="epp", bufs=3))
        yp = ctx.enter_context(tc.tile_pool(name="yp", bufs=16))
        rp = ctx.enter_context(tc.tile_pool(name="rp", bufs=4))
        op = ctx.enter_context(tc.tile_pool(name="op", bufs=2))
        ps = ctx.enter_context(tc.tile_pool(name="ps", bufs=3, space="PSUM"))
        psp = ctx.enter_context(tc.tile_pool(name="psp", bufs=1, space="PSUM"))
        pyp = ctx.enter_context(tc.tile_pool(name="pyp", bufs=2, space="PSUM"))
        ptp = ctx.enter_context(tc.tile_pool(name="ptp", bufs=1, space="PSUM"))

        # ---- persistent loads ----
        xt = []
        for k in range(KC):
            t_ = pers.tile([128, T], F32, tag=f"xt{k}")
            nc.sync.dma_start(t_[:], xT_d[128 * k:128 * k + 128, :])
            xt.append(t_)
        wv = []
        wp = []
        for k in range(KC):
            t_ = pers.tile([128, C], F32, tag=f"wv{k}")
            nc.sync.dma_start(t_[:], wv_d[128 * k:128 * k + 128, :])
            wv.append(t_)
            t_ = pers.tile([128, C], F32, tag=f"wp{k}")
            nc.sync.dma_start(t_[:], wp_d[128 * k:128 * k + 128, :])
            wp.append(t_)
        bqk = pers.tile([128, 12], F32, tag="bqk")
        nc.sync.dma_start(bqk[:], bqk_d[:])
        bv = pers.tile([128, C], F32, tag="bv")
        nc.sync.dma_start(bv[:], bv_d[:])
        bp = pers.tile([128, C], F32, tag="bp")
        nc.sync.dma_start(bp[:], bp_d[:])
        tri = pers.tile([128, 128], F32, tag="tri")
        nc.sync.dma_start(tri[:], tri_d[:])
        ident = pers.tile([128, 128], F32, tag="ident")
        nc.sync.dma_start(ident[:], id_d[:])
        vc = pers.tile([PFX, H, 65], F32, tag="vc")
        nc.sync.dma_start(vc[:], vc_d[:])

        yT = [pers.tile([128, T], F32, tag=f"yT{p}", name=f"yT{p}")
              for p in range(NPAIR)]

        # ---- v projection: natural layout with interleaved ones column ----
        vt = []
        for mt in range(TCH):
            v_ = pers.tile([128, H, 65], F32, tag=f"v{mt}")
            nc.vector.memset(v_[:, :, 64:65], 1.0)
            for n0, nsz in ((0, 512), (512, 256)):
                pv = ps.tile([128, 512], F32, tag="ps")
                for k in range(KC):
                    nc.tensor.matmul(pv[:, :nsz],
                                     xt[k][:, 128 * mt:128 * mt + 128],
                                     wv[k][:, n0:n0 + nsz],
                                     start=(k == 0), stop=(k == KC - 1))
                h0, hn = n0 // 64, nsz // 64
                nc.vector.tensor_add(
                    v_[:, h0:h0 + hn, 0:64],
                    pv[:, :nsz].rearrange("a (h d) -> a h d", d=64),
                    bv[:, n0:n0 + nsz].rearrange("a (h d) -> a h d", d=64))
            vt.append(v_)

        # ---- per head-pair attention ----
        for p in range(NPAIR):
            # weights for this pair: [128, 2, 128] per k-chunk (q col, k col)
            wq = []
            for k in range(KC):
                t_ = wqkp.tile([128, 2, 128], F32, tag="wqk")
                src = wqk_d[128 * k:128 * k + 128, :].rearrange(
                    "a (s b) -> a s b", s=2)[:, :, 128 * p:128 * p + 128]
                nc.sync.dma_start(t_[:], src)
                wq.append(t_)

            qT = qkp.tile([128, T], F32, tag="qT")
            kT = qkp.tile([128, T + PFX], F32, tag="kT")
            for half, dst in ((0, qT), (1, kT)):
                for w in range(NW):
                    pq = ps.tile([128, 512], F32, tag="ps")
                    for k in range(KC):
                        nc.tensor.matmul(pq[:], wq[k][:, half, :],
                                         xt[k][:, W * w:W * w + W],
                                         start=(k == 0), stop=(k == KC - 1))
                    nc.vector.tensor_scalar_add(
                        dst[:, W * w:W * w + W], pq[:],
                        bqk[:, 6 * half + p:6 * half + p + 1])
            nc.sync.dma_start(kT[:, T:T + PFX], kTc_d[128 * p:128 * p + 128, :])

            ysb = [yp.tile([128, 128], F32, tag="ysb", name=f"ysb{p}_{i}")
                   for i in range(TCH)]

            for s in range(2):
                h = 2 * p + s
                qs = qT[64 * s:64 * s + 64, :]
                ks = kT[64 * s:64 * s + 64, :]
                et = {}
                etp = {}
                # scores + exp (+ causal band mask) per window
                for c in range(NW):
                    for r in range(4 * c + 4):
                        e_ = ep.tile([128, W], F32, tag="et")
                        pss = ps.tile([128, 512], F32, tag="ps")
                        if r >= 4 * c:  # diagonal band tile
                            j0 = 128 * r - W * c
                            nc.tensor.matmul(pss[:, j0:W], ks[:, 128 * r:128 * r + 128],
                                             qs[:, W * c + j0:W * (c + 1)],
                                             start=True, stop=True)
                            nc.scalar.activation(e_[:, j0:W], pss[:, j0:W], EXP,
                                                 scale=float(SCALE))
                            nc.vector.tensor_mul(e_[:, j0:j0 + 128],
                                                 e_[:, j0:j0 + 128], tri[:])
                        else:
                            nc.tensor.matmul(pss[:], ks[:, 128 * r:128 * r + 128],
                                             qs[:, W * c:W * (c + 1)],
                                             start=True, stop=True)
                            nc.scalar.activation(e_[:], pss[:], EXP,
                                                 scale=float(SCALE))
                        et[(c, r)] = e_
                    pp = psp.tile([PFX, 512], F32, tag="psp")
                    nc.tensor.matmul(pp[:], ks[:, T:T + PFX],
                                     qs[:, W * c:W * (c + 1)], start=True, stop=True)
                    ep_ = epp.tile([PFX, W], F32, tag="etp")
                    nc.scalar.activation(ep_[:], pp[:], EXP, scale=float(SCALE))
                    etp[c] = ep_
                # AV + normalize
                for c in range(NW):
                    for mloc in range(4):
                        m = 4 * c + mloc
                        py = pyp.tile([128, 65], F32, tag="py")
                        for r in range(m + 1):
                            nc.tensor.matmul(py[:],
                                             et[(c, r)][:, 128 * mloc:128 * mloc + 128],
                                             vt[r][:, h, :],
                                             start=(r == 0), stop=False)
                        nc.tensor.matmul(py[:],
                                         etp[c][:, 128 * mloc:128 * mloc + 128],
                                         vc[:, h, :], start=False, stop=True)
                        rc = rp.tile([128, 1], F32, tag="rc")
                        nc.vector.reciprocal(rc[:], py[:, 64:65])
                        nc.vector.tensor_scalar_mul(
                            ysb[m][:, 64 * s:64 * s + 64], py[:, 0:64], rc[:])

            # transpose y (both heads at once) into yT pair tile
            for m in range(TCH):
                pt = ptp.tile([128, 128], F32, tag="pt")
                nc.tensor.transpose(pt[:], ysb[m][:], ident[:])
                nc.vector.tensor_copy(yT[p][:, 128 * m:128 * m + 128], pt[:])

        # ---- output projection ----
        for mt in range(TCH):
            osb = op.tile([128, C], F32, tag="osb")
            for n0, nsz in ((0, 512), (512, 256)):
                po = ps.tile([128, 512], F32, tag="ps")
                for kp in range(NPAIR):
                    nc.tensor.matmul(po[:, :nsz],
                                     yT[kp][:, 128 * mt:128 * mt + 128],
                                     wp[kp][:, n0:n0 + nsz],
                                     start=(kp == 0), stop=(kp == NPAIR - 1))
                nc.vector.tensor_add(osb[:, n0:n0 + nsz], po[:, :nsz],
                                     bp[:, n0:n0 + nsz])
            nc.sync.dma_start(out_d[128 * mt:128 * mt + 128, :], osb[:])

    nc.finalize()
    return nc


def _prep_inputs(x, kv_cvec, w_attn, b_attn, w_proj, b_proj):
    x = np.asarray(x, np.float32)
    kv_cvec = np.asarray(kv_cvec, np.float32)
    w_attn = np.asarray(w_attn, np.float32)
    b_attn = np.asarray(b_attn, np.float32)
    w_proj = np.asarray(w_proj, np.float32)
    b_proj = np.asarray(b_proj, np.float32)

    shared = {
        "w_qk": np.ascontiguousarray(w_attn[:, :2 * C]),
        "w_v": np.ascontiguousarray(w_attn[:, 2 * C:]),
        "w_p": np.ascontiguousarray(w_proj),
        "b_qk": np.ascontiguousarray(b_attn[:2 * C].reshape(12, 128).T),
        "bv_bc": np.ascontiguousarray(
            np.broadcast_to(b_attn[2 * C:], (128, C))),
        "bp_bc": np.ascontiguousarray(np.broadcast_to(b_proj, (128, C))),
        "tri": (np.arange(128)[:, None] <= np.arange(128)[None, :]
                ).astype(np.float32),
        "ident": np.eye(128, dtype=np.float32),
    }
    in_maps = []
    for b in range(N_CORES):
        vc_aug = np.zeros((PFX, H, 65), np.float32)
        vc_aug[:, :, :64] = kv_cvec[b][:, C:].reshape(PFX, H, D)
        vc_aug[:, :, 64] = 1.0
        m = dict(shared)
        m["xT"] = np.ascontiguousarray(x[b].T)
        m["kTc"] = np.ascontiguousarray(kv_cvec[b][:, :C].T)
        m["vc_aug"] = vc_aug
        in_maps.append(m)
    return in_maps


_NC_CACHE = {}


def run_hw(trace=False, **inputs):
    """Build+compile+run on 8 NeuronCores; returns (out [8,1024,768], results)."""
    if "nc" not in _NC_CACHE:
        _NC_CACHE["nc"] = _build()
    nc = _NC_CACHE["nc"]
    in_maps = _prep_inputs(**inputs)
    res = run_bass_kernel_spmd(nc, in_maps, list(range(N_CORES)), trace=trace)
    out = np.stack([res.results[b]["out"] for b in range(N_CORES)])
    return out, res


def kernel(**inputs):
    out, _ = run_hw(trace=False, **inputs)
    return out


# revision 4
# speedup vs baseline: 3.1105x; 3.1105x over previous
"""Causal self-attention (12 heads, T=1024, C=768, prefix P=4) on 8 TRN2 cores.

Sharding: data-parallel over batch B=8 -> one batch element per NeuronCore.
No collectives. Weights are replicated to every core.

Per-core kernel (all fp32):
  qkv projection split by destination layout:
    qT,kT  [128, T] per head-pair (transposed layout) = w_attn_slice.T @ xT
    v      [T, 12*65] natural layout (65th col per head = 1.0 for the
           softmax denominator), = xT_slice.T @ w_v
  prefix k/v (4 positions) are appended at the END of the kv axis, so the
  causal structure is block lower-triangular in (kv-chunk, t-chunk) space:
    scores^T tile (r, window c): psum = kT_slice.T @ qT_window  [128kv, 512t]
    e = exp(0.125 * psum)  (no max subtraction: |scores| ~ 2)
    diagonal band tiles multiplied by a 128x128 triangular 0/1 mask;
    fully-masked columns are never computed nor read.
  AV: y[tchunk] = sum_r e^T(r).T @ v_aug(r)  -> psum [128t, 65]
    col 64 = softmax denominator; normalize via DVE reciprocal +
    per-partition tensor_scalar_mul.  Two heads share a [128,128] y tile,
    one PE transpose each -> yT pair tiles [128, T].
  out = yT.T @ w_proj + b_proj  -> [T, 768] -> DMA out.
"""

import numpy as np
from contextlib import ExitStack

import concourse.bass as bass
import concourse.mybir as mybir
import concourse.tile as tile
from concourse import bacc
from concourse.bass_utils import run_bass_kernel_spmd

F32 = mybir.dt.float32
F32R = mybir.dt.float32r
BF16 = mybir.dt.bfloat16
N_CORES = 8
T, C, H, D, PFX = 1024, 768, 12, 64, 4
NPAIR = H // 2          # 6 head pairs
KC = C // 128           # 6 contraction chunks
W = 512                 # T window for scores
NW = T // W             # 2 windows
TCH = T // 128          # 8 T chunks
EXP = mybir.ActivationFunctionType.Exp
SCALE = 1.0 / np.sqrt(D)


def _build():
    nc = bacc.Bacc("TRN2", target_bir_lowering=False, debug=False,
                   num_devices=N_CORES)
    xT_d = nc.declare_dram_parameter("xT", [C, T], F32, isOutput=False)
    wqk_d = nc.declare_dram_parameter("w_qk", [C, 2 * C], BF16, isOutput=False)
    wv_d = nc.declare_dram_parameter("w_v", [C, C], BF16, isOutput=False)
    wp_d = nc.declare_dram_parameter("w_p", [C, C], F32, isOutput=False)
    bqk_d = nc.declare_dram_parameter("b_qk", [128, 12], F32, isOutput=False)
    bv_d = nc.declare_dram_parameter("bv_bc", [128, C], F32, isOutput=False)
    bp_d = nc.declare_dram_parameter("bp_bc", [128, C], F32, isOutput=False)
    kTc_d = nc.declare_dram_parameter("kTc", [C, PFX], BF16, isOutput=False)
    vc_d = nc.declare_dram_parameter("vc_aug", [PFX, H, 65], BF16, isOutput=False)
    tri_d = nc.declare_dram_parameter("tri", [128, 128], BF16, isOutput=False)
    id_d = nc.declare_dram_parameter("ident", [128, 128], F32, isOutput=False)
    out_d = nc.declare_dram_parameter("out", [T, C], F32, isOutput=True)

    with tile.TileContext(nc) as tc, ExitStack() as ctx:
        pers = ctx.enter_context(tc.tile_pool(name="pers", bufs=1))
        wqkp = ctx.enter_context(tc.tile_pool(name="wqkp", bufs=12))
        qkp = ctx.enter_context(tc.tile_pool(name="qkp", bufs=2))
        ep = ctx.enter_context(tc.tile_pool(name="ep", bufs=14))
        epp = ctx.enter_context(tc.tile_pool(name="epp", bufs=3))
        yp = ctx.enter_context(tc.tile_pool(name="yp", bufs=16))
        rp = ctx.enter_context(tc.tile_pool(name="rp", bufs=4))
        op = ctx.enter_context(tc.tile_pool(name="op", bufs=2))
        ps = ctx.enter_context(tc.tile_pool(name="ps", bufs=3, space="PSUM"))
        psp = ctx.enter_context(tc.tile_pool(name="psp", bufs=1, space="PSUM"))
        pyp = ctx.enter_context(tc.tile_pool(name="pyp", bufs=2, space="PSUM"))
        ptp = ctx.enter_context(tc.tile_pool(name="ptp", bufs=1, space="PSUM"))

        # ---- persistent loads ----
        xt = []
        for k in range(KC):
            t_ = pers.tile([128, T], F32, tag=f"xt{k}")
            nc.sync.dma_start(t_[:], xT_d[128 * k:128 * k + 128, :])
            xt.append(t_)
        wv = []
        wp = []
        for k in range(KC):
            t_ = pers.tile([128, C], BF16, tag=f"wv{k}")
            nc.sync.dma_start(t_[:], wv_d[128 * k:128 * k + 128, :])
            wv.append(t_)
            t_ = pers.tile([128, C], F32R, tag=f"wp{k}")
            nc.sync.dma_start(t_[:], wp_d[128 * k:128 * k + 128, :].bitcast(F32R))
            wp.append(t_)
        bqk = pers.tile([128, 12], F32, tag="bqk")
        nc.sync.dma_start(bqk[:], bqk_d[:])
        bv = pers.tile([128, C], F32, tag="bv")
        nc.sync.dma_start(bv[:], bv_d[:])
        bp = pers.tile([128, C], F32, tag="bp")
        nc.sync.dma_start(bp[:], bp_d[:])
        tri = pers.tile([128, 128], BF16, tag="tri")
        nc.sync.dma_start(tri[:], tri_d[:])
        ident = pers.tile([128, 128], F32, tag="ident")
        nc.sync.dma_start(ident[:], id_d[:])
        vc = pers.tile([PFX, H, 65], BF16, tag="vc")
        nc.sync.dma_start(vc[:], vc_d[:])

        yT = [pers.tile([128, T], F32R, tag=f"yT{p}", name=f"yT{p}")
              for p in range(NPAIR)]
        xtb = []
        for k in range(KC):
            t_ = pers.tile([128, T], BF16, tag=f"xtb{k}")
            nc.vector.tensor_copy(t_[:], xt[k][:])
            xtb.append(t_)

        # ---- v projection: natural layout with interleaved ones column ----
        vt = []
        for mt in range(TCH):
            v_ = pers.tile([128, H, 65], BF16, tag=f"v{mt}")
            nc.vector.memset(v_[:, :, 64:65], 1.0)
            for n0, nsz in ((0, 512), (512, 256)):
                pv = ps.tile([128, 512], F32, tag="ps")
                for k in range(KC):
                    nc.tensor.matmul(pv[:, :nsz],
                                     xtb[k][:, 128 * mt:128 * mt + 128],
                                     wv[k][:, n0:n0 + nsz],
                                     start=(k == 0), stop=(k == KC - 1))
                h0, hn = n0 // 64, nsz // 64
                nc.vector.tensor_add(
                    v_[:, h0:h0 + hn, 0:64],
                    pv[:, :nsz].rearrange("a (h d) -> a h d", d=64),
                    bv[:, n0:n0 + nsz].rearrange("a (h d) -> a h d", d=64))
            vt.append(v_)

        # ---- per head-pair attention ----
        for p in range(NPAIR):
            # weights for this pair: [128, 2, 128] per k-chunk (q col, k col)
            wq = []
            for k in range(KC):
                t_ = wqkp.tile([128, 2, 128], BF16, tag="wqk")
                src = wqk_d[128 * k:128 * k + 128, :].rearrange(
                    "a (s b) -> a s b", s=2)[:, :, 128 * p:128 * p + 128]
                nc.sync.dma_start(t_[:], src)
                wq.append(t_)

            qT = qkp.tile([128, T], BF16, tag="qT")
            kT = qkp.tile([128, T + PFX], BF16, tag="kT")
            for half, dst in ((0, qT), (1, kT)):
                for w in range(NW):
                    pq = ps.tile([128, 512], F32, tag="ps")
                    for k in range(KC):
                        nc.tensor.matmul(pq[:], wq[k][:, half, :],
                                         xtb[k][:, W * w:W * w + W],
                                         start=(k == 0), stop=(k == KC - 1))
                    nc.vector.tensor_scalar_add(
                        dst[:, W * w:W * w + W], pq[:],
                        bqk[:, 6 * half + p:6 * half + p + 1])
            nc.sync.dma_start(kT[:, T:T + PFX], kTc_d[128 * p:128 * p + 128, :])

            ysb = [yp.tile([128, 128], F32, tag="ysb", name=f"ysb{p}_{i}")
                   for i in range(TCH)]

            for s in range(2):
                h = 2 * p + s
                qs = qT[64 * s:64 * s + 64, :]
                ks = kT[64 * s:64 * s + 64, :]
                et = {}
                etp = {}
                # scores + exp (+ causal band mask) per window
                for c in range(NW):
                    for r in range(4 * c + 4):
                        e_ = ep.tile([128, W], BF16, tag="et")
                        pss = ps.tile([128, 512], F32, tag="ps")
                        if r >= 4 * c:  # diagonal band tile
                            j0 = 128 * r - W * c
                            nc.tensor.matmul(pss[:, j0:W], ks[:, 128 * r:128 * r + 128],
                                             qs[:, W * c + j0:W * (c + 1)],
                                             start=True, stop=True)
                            nc.scalar.activation(e_[:, j0:W], pss[:, j0:W], EXP,
                                                 scale=float(SCALE))
                            nc.vector.tensor_mul(e_[:, j0:j0 + 128],
                                                 e_[:, j0:j0 + 128], tri[:])
                        else:
                            nc.tensor.matmul(pss[:], ks[:, 128 * r:128 * r + 128],
                                             qs[:, W * c:W * (c + 1)],
                                             start=True, stop=True)
                            nc.scalar.activation(e_[:], pss[:], EXP,
                                                 scale=float(SCALE))
                        et[(c, r)] = e_
                    pp = psp.tile([PFX, 512], F32, tag="psp")
                    nc.tensor.matmul(pp[:], ks[:, T:T + PFX],
                                     qs[:, W * c:W * (c + 1)], start=True, stop=True)
                    ep_ = epp.tile([PFX, W], BF16, tag="etp")
                    nc.scalar.activation(ep_[:], pp[:], EXP, scale=float(SCALE))
                    etp[c] = ep_
                # AV + normalize
                for c in range(NW):
                    for mloc in range(4):
                        m = 4 * c + mloc
                        py = pyp.tile([128, 65], F32, tag="py")
                        for r in range(m + 1):
                            nc.tensor.matmul(py[:],
                                             et[(c, r)][:, 128 * mloc:128 * mloc + 128],
                                             vt[r][:, h, :],
                                             start=(r == 0), stop=False)
                        nc.tensor.matmul(py[:],
                                         etp[c][:, 128 * mloc:128 * mloc + 128],
                                         vc[:, h, :], start=False, stop=True)
                        rc = rp.tile([128, 1], F32, tag="rc")
                        nc.vector.reciprocal(rc[:], py[:, 64:65])
                        nc.vector.tensor_scalar_mul(
                            ysb[m][:, 64 * s:64 * s + 64], py[:, 0:64], rc[:])

            # transpose y (both heads at once) into yT pair tile
            for m in range(TCH):
                pt = ptp.tile([128, 128], F32, tag="pt")
                nc.tensor.transpose(pt[:], ysb[m][:], ident[:])
                nc.vector.tensor_copy(yT[p][:, 128 * m:128 * m + 128], pt[:])

        # ---- output projection ----
        for mt in range(TCH):
            osb = op.tile([128, C], F32, tag="osb")
            for n0, nsz in ((0, 512), (512, 256)):
                po = ps.tile([128, 512], F32, tag="ps")
                for kp in range(NPAIR):
                    nc.tensor.matmul(po[:, :nsz],
                                     yT[kp][:, 128 * mt:128 * mt + 128],
                                     wp[kp][:, n0:n0 + nsz],
                                     start=(kp == 0), stop=(kp == NPAIR - 1))
                nc.vector.tensor_add(osb[:, n0:n0 + nsz], po[:, :nsz],
                                     bp[:, n0:n0 + nsz])
            nc.sync.dma_start(out_d[128 * mt:128 * mt + 128, :], osb[:])

    nc.finalize()
    return nc


def _prep_inputs(x, kv_cvec, w_attn, b_attn, w_proj, b_proj):
    x = np.asarray(x, np.float32)
    kv_cvec = np.asarray(kv_cvec, np.float32)
    w_attn = np.asarray(w_attn, np.float32)
    b_attn = np.asarray(b_attn, np.float32)
    w_proj = np.asarray(w_proj, np.float32)
    b_proj = np.asarray(b_proj, np.float32)

    import ml_dtypes
    shared = {
        "w_qk": np.ascontiguousarray(w_attn[:, :2 * C]).astype(ml_dtypes.bfloat16),
        "w_v": np.ascontiguousarray(w_attn[:, 2 * C:]).astype(ml_dtypes.bfloat16),
        "w_p": np.ascontiguousarray(w_proj),
        "b_qk": np.ascontiguousarray(b_attn[:2 * C].reshape(12, 128).T),
        "bv_bc": np.ascontiguousarray(
            np.broadcast_to(b_attn[2 * C:], (128, C))),
        "bp_bc": np.ascontiguousarray(np.broadcast_to(b_proj, (128, C))),
        "tri": (np.arange(128)[:, None] <= np.arange(128)[None, :]
                ).astype(ml_dtypes.bfloat16),
        "ident": np.eye(128, dtype=np.float32),
    }
    in_maps = []
    for b in range(N_CORES):
        vc_aug = np.zeros((PFX, H, 65), np.float32)
        vc_aug[:, :, :64] = kv_cvec[b][:, C:].reshape(PFX, H, D)
        vc_aug[:, :, 64] = 1.0
        m = dict(shared)
        m["xT"] = np.ascontiguousarray(x[b].T)
        m["kTc"] = np.ascontiguousarray(kv_cvec[b][:, :C].T
                                        ).astype(ml_dtypes.bfloat16)
        m["vc_aug"] = vc_aug.astype(ml_dtypes.bfloat16)
        in_maps.append(m)
    return in_maps


_NC_CACHE = {}


def run_hw(trace=False, **inputs):
    """Build+compile+run on 8 NeuronCores; returns (out [8,1024,768], results)."""
    if "nc" not in _NC_CACHE:
        _NC_CACHE["nc"] = _build()
    nc = _NC_CACHE["nc"]
    in_maps = _prep_inputs(**inputs)
    res = run_bass_kernel_spmd(nc, in_maps, list(range(N_CORES)), trace=trace)
    out = np.stack([res.results[b]["out"] for b in range(N_CORES)])
    return out, res


def kernel(**inputs):
    out, _ = run_hw(trace=False, **inputs)
    return out


# revision 5
# speedup vs baseline: 3.1173x; 1.0022x over previous
"""Causal self-attention (12 heads, T=1024, C=768, prefix P=4) on 8 TRN2 cores.

Sharding: data-parallel over batch B=8 -> one batch element per NeuronCore.
No collectives. Weights are replicated to every core.

Per-core kernel (all fp32):
  qkv projection split by destination layout:
    qT,kT  [128, T] per head-pair (transposed layout) = w_attn_slice.T @ xT
    v      [T, 12*65] natural layout (65th col per head = 1.0 for the
           softmax denominator), = xT_slice.T @ w_v
  prefix k/v (4 positions) are appended at the END of the kv axis, so the
  causal structure is block lower-triangular in (kv-chunk, t-chunk) space:
    scores^T tile (r, window c): psum = kT_slice.T @ qT_window  [128kv, 512t]
    e = exp(0.125 * psum)  (no max subtraction: |scores| ~ 2)
    diagonal band tiles multiplied by a 128x128 triangular 0/1 mask;
    fully-masked columns are never computed nor read.
  AV: y[tchunk] = sum_r e^T(r).T @ v_aug(r)  -> psum [128t, 65]
    col 64 = softmax denominator; normalize via DVE reciprocal +
    per-partition tensor_scalar_mul.  Two heads share a [128,128] y tile,
    one PE transpose each -> yT pair tiles [128, T].
  out = yT.T @ w_proj + b_proj  -> [T, 768] -> DMA out.
"""

import numpy as np
from contextlib import ExitStack

import concourse.bass as bass
import concourse.mybir as mybir
import concourse.tile as tile
from concourse import bacc
from concourse.bass_utils import run_bass_kernel_spmd

F32 = mybir.dt.float32
F32R = mybir.dt.float32r
F16 = mybir.dt.float16
N_CORES = 8
T, C, H, D, PFX = 1024, 768, 12, 64, 4
NPAIR = H // 2          # 6 head pairs
KC = C // 128           # 6 contraction chunks
W = 512                 # T window for scores
NW = T // W             # 2 windows
TCH = T // 128          # 8 T chunks
EXP = mybir.ActivationFunctionType.Exp
SCALE = 1.0 / np.sqrt(D)


def _build():
    nc = bacc.Bacc("TRN2", target_bir_lowering=False, debug=False,
                   num_devices=N_CORES)
    xT_d = nc.declare_dram_parameter("xT", [C, T], F32, isOutput=False)
    wqk_d = nc.declare_dram_parameter("w_qk", [C, 2 * C], F16, isOutput=False)
    wv_d = nc.declare_dram_parameter("w_v", [C, C], F16, isOutput=False)
    wp_d = nc.declare_dram_parameter("w_p", [C, C], F32, isOutput=False)
    bqk_d = nc.declare_dram_parameter("b_qk", [128, 12], F32, isOutput=False)
    bv_d = nc.declare_dram_parameter("bv_bc", [128, C], F32, isOutput=False)
    bp_d = nc.declare_dram_parameter("bp_bc", [128, C], F32, isOutput=False)
    kTc_d = nc.declare_dram_parameter("kTc", [C, PFX], F16, isOutput=False)
    vc_d = nc.declare_dram_parameter("vc_aug", [PFX, H, 65], F16, isOutput=False)
    tri_d = nc.declare_dram_parameter("tri", [128, 128], F16, isOutput=False)
    id_d = nc.declare_dram_parameter("ident", [128, 128], F32, isOutput=False)
    out_d = nc.declare_dram_parameter("out", [T, C], F32, isOutput=True)

    with tile.TileContext(nc) as tc, ExitStack() as ctx:
        pers = ctx.enter_context(tc.tile_pool(name="pers", bufs=1))
        wqkp = ctx.enter_context(tc.tile_pool(name="wqkp", bufs=12))
        qkp = ctx.enter_context(tc.tile_pool(name="qkp", bufs=2))
        ep = ctx.enter_context(tc.tile_pool(name="ep", bufs=14))
        epp = ctx.enter_context(tc.tile_pool(name="epp", bufs=3))
        yp = ctx.enter_context(tc.tile_pool(name="yp", bufs=16))
        rp = ctx.enter_context(tc.tile_pool(name="rp", bufs=4))
        op = ctx.enter_context(tc.tile_pool(name="op", bufs=2))
        ps = ctx.enter_context(tc.tile_pool(name="ps", bufs=3, space="PSUM"))
        psp = ctx.enter_context(tc.tile_pool(name="psp", bufs=1, space="PSUM"))
        pyp = ctx.enter_context(tc.tile_pool(name="pyp", bufs=2, space="PSUM"))
        ptp = ctx.enter_context(tc.tile_pool(name="ptp", bufs=1, space="PSUM"))

        # ---- persistent loads ----
        xt = []
        for k in range(KC):
            t_ = pers.tile([128, T], F32, tag=f"xt{k}")
            nc.sync.dma_start(t_[:], xT_d[128 * k:128 * k + 128, :])
            xt.append(t_)
        wv = []
        wp = []
        for k in range(KC):
            t_ = pers.tile([128, C], F16, tag=f"wv{k}")
            nc.sync.dma_start(t_[:], wv_d[128 * k:128 * k + 128, :])
            wv.append(t_)
            t_ = pers.tile([128, C], F32R, tag=f"wp{k}")
            nc.sync.dma_start(t_[:], wp_d[128 * k:128 * k + 128, :].bitcast(F32R))
            wp.append(t_)
        bqk = pers.tile([128, 12], F32, tag="bqk")
        nc.sync.dma_start(bqk[:], bqk_d[:])
        bv = pers.tile([128, C], F32, tag="bv")
        nc.sync.dma_start(bv[:], bv_d[:])
        bp = pers.tile([128, C], F32, tag="bp")
        nc.sync.dma_start(bp[:], bp_d[:])
        tri = pers.tile([128, 128], F16, tag="tri")
        nc.sync.dma_start(tri[:], tri_d[:])
        ident = pers.tile([128, 128], F32, tag="ident")
        nc.sync.dma_start(ident[:], id_d[:])
        vc = pers.tile([PFX, H, 65], F16, tag="vc")
        nc.sync.dma_start(vc[:], vc_d[:])

        yT = [pers.tile([128, T], F32R, tag=f"yT{p}", name=f"yT{p}")
              for p in range(NPAIR)]
        xtb = []
        for k in range(KC):
            t_ = pers.tile([128, T], F16, tag=f"xtb{k}")
            nc.vector.tensor_copy(t_[:], xt[k][:])
            xtb.append(t_)

        # ---- v projection: natural layout with interleaved ones column ----
        vt = []
        for mt in range(TCH):
            v_ = pers.tile([128, H, 65], F16, tag=f"v{mt}")
            nc.vector.memset(v_[:, :, 64:65], 1.0)
            for n0, nsz in ((0, 512), (512, 256)):
                pv = ps.tile([128, 512], F32, tag="ps")
                for k in range(KC):
                    nc.tensor.matmul(pv[:, :nsz],
                                     xtb[k][:, 128 * mt:128 * mt + 128],
                                     wv[k][:, n0:n0 + nsz],
                                     start=(k == 0), stop=(k == KC - 1))
                h0, hn = n0 // 64, nsz // 64
                nc.vector.tensor_add(
                    v_[:, h0:h0 + hn, 0:64],
                    pv[:, :nsz].rearrange("a (h d) -> a h d", d=64),
                    bv[:, n0:n0 + nsz].rearrange("a (h d) -> a h d", d=64))
            vt.append(v_)

        # ---- per head-pair attention ----
        for p in range(NPAIR):
            # weights for this pair: [128, 2, 128] per k-chunk (q col, k col)
            wq = []
            for k in range(KC):
                t_ = wqkp.tile([128, 2, 128], F16, tag="wqk")
                src = wqk_d[128 * k:128 * k + 128, :].rearrange(
                    "a (s b) -> a s b", s=2)[:, :, 128 * p:128 * p + 128]
                nc.sync.dma_start(t_[:], src)
                wq.append(t_)

            qT = qkp.tile([128, T], F16, tag="qT")
            kT = qkp.tile([128, T + PFX], F16, tag="kT")
            for half, dst in ((0, qT), (1, kT)):
                for w in range(NW):
                    pq = ps.tile([128, 512], F32, tag="ps")
                    for k in range(KC):
                        nc.tensor.matmul(pq[:], wq[k][:, half, :],
                                         xtb[k][:, W * w:W * w + W],
                                         start=(k == 0), stop=(k == KC - 1))
                    nc.vector.tensor_scalar_add(
                        dst[:, W * w:W * w + W], pq[:],
                        bqk[:, 6 * half + p:6 * half + p + 1])
            nc.sync.dma_start(kT[:, T:T + PFX], kTc_d[128 * p:128 * p + 128, :])

            ysb = [yp.tile([128, 128], F32, tag="ysb", name=f"ysb{p}_{i}")
                   for i in range(TCH)]

            for s in range(2):
                h = 2 * p + s
                qs = qT[64 * s:64 * s + 64, :]
                ks = kT[64 * s:64 * s + 64, :]
                et = {}
                etp = {}
                # scores + exp (+ causal band mask) per window
                for c in range(NW):
                    for r in range(4 * c + 4):
                        e_ = ep.tile([128, W], F16, tag="et")
                        pss = ps.tile([128, 512], F32, tag="ps")
                        if r >= 4 * c:  # diagonal band tile
                            j0 = 128 * r - W * c
                            nc.tensor.matmul(pss[:, j0:W], ks[:, 128 * r:128 * r + 128],
                                             qs[:, W * c + j0:W * (c + 1)],
                                             start=True, stop=True)
                            nc.scalar.activation(e_[:, j0:W], pss[:, j0:W], EXP,
                                                 scale=float(SCALE))
                            nc.vector.tensor_mul(e_[:, j0:j0 + 128],
                                                 e_[:, j0:j0 + 128], tri[:])
                        else:
                            nc.tensor.matmul(pss[:], ks[:, 128 * r:128 * r + 128],
                                             qs[:, W * c:W * (c + 1)],
                                             start=True, stop=True)
                            nc.scalar.activation(e_[:], pss[:], EXP,
                                                 scale=float(SCALE))
                        et[(c, r)] = e_
                    pp = psp.tile([PFX, 512], F32, tag="psp")
                    nc.tensor.matmul(pp[:], ks[:, T:T + PFX],
                                     qs[:, W * c:W * (c + 1)], start=True, stop=True)
                    ep_ = epp.tile([PFX, W], F16, tag="etp")
                    nc.scalar.activation(ep_[:], pp[:], EXP, scale=float(SCALE))
                    etp[c] = ep_
                # AV + normalize
                for c in range(NW):
                    for mloc in range(4):
                        m = 4 * c + mloc
                        py = pyp.tile([128, 65], F32, tag="py")
                        for r in range(m + 1):
                            nc.tensor.matmul(py[:],
                                             et[(c, r)][:, 128 * mloc:128 * mloc + 128],
                                             vt[r][:, h, :],
                                             start=(r == 0), stop=False)
                        nc.tensor.matmul(py[:],
                                         etp[c][:, 128 * mloc:128 * mloc + 128],
                                         vc[:, h, :], start=False, stop=True)
                        rc = rp.tile([128, 1], F32, tag="rc")
                        nc.vector.reciprocal(rc[:], py[:, 64:65])
                        nc.vector.tensor_scalar_mul(
                            ysb[m][:, 64 * s:64 * s + 64], py[:, 0:64], rc[:])

            # transpose y (both heads at once) into yT pair tile
            for m in range(TCH):
                pt = ptp.tile([128, 128], F32, tag="pt")
                nc.tensor.transpose(pt[:], ysb[m][:], ident[:])
                nc.vector.tensor_copy(yT[p][:, 128 * m:128 * m + 128], pt[:])

        # ---- output projection ----
        for mt in range(TCH):
            osb = op.tile([128, C], F32, tag="osb")
            for n0, nsz in ((0, 512), (512, 256)):
                po = ps.tile([128, 512], F32, tag="ps")
                for kp in range(NPAIR):
                    nc.tensor.matmul(po[:, :nsz],
                                     yT[kp][:, 128 * mt:128 * mt + 128],
                                     wp[kp][:, n0:n0 + nsz],
                                     start=(kp == 0), stop=(kp == NPAIR - 1))
                nc.vector.tensor_add(osb[:, n0:n0 + nsz], po[:, :nsz],
                                     bp[:, n0:n0 + nsz])
            nc.sync.dma_start(out_d[128 * mt:128 * mt + 128, :], osb[:])

    nc.finalize()
    return nc


def _prep_inputs(x, kv_cvec, w_attn, b_attn, w_proj, b_proj):
    x = np.asarray(x, np.float32)
    kv_cvec = np.asarray(kv_cvec, np.float32)
    w_attn = np.asarray(w_attn, np.float32)
    b_attn = np.asarray(b_attn, np.float32)
    w_proj = np.asarray(w_proj, np.float32)
    b_proj = np.asarray(b_proj, np.float32)

    shared = {
        "w_qk": np.ascontiguousarray(w_attn[:, :2 * C]).astype(np.float16),
        "w_v": np.ascontiguousarray(w_attn[:, 2 * C:]).astype(np.float16),
        "w_p": np.ascontiguousarray(w_proj),
        "b_qk": np.ascontiguousarray(b_attn[:2 * C].reshape(12, 128).T),
        "bv_bc": np.ascontiguousarray(
            np.broadcast_to(b_attn[2 * C:], (128, C))),
        "bp_bc": np.ascontiguousarray(np.broadcast_to(b_proj, (128, C))),
        "tri": (np.arange(128)[:, None] <= np.arange(128)[None, :]
                ).astype(np.float16),
        "ident": np.eye(128, dtype=np.float32),
    }
    in_maps = []
    for b in range(N_CORES):
        vc_aug = np.zeros((PFX, H, 65), np.float32)
        vc_aug[:, :, :64] = kv_cvec[b][:, C:].reshape(PFX, H, D)
        vc_aug[:, :, 64] = 1.0
        m = dict(shared)
        m["xT"] = np.ascontiguousarray(x[b].T)
        m["kTc"] = np.ascontiguousarray(kv_cvec[b][:, :C].T
                                        ).astype(np.float16)
        m["vc_aug"] = vc_aug.astype(np.float16)
        in_maps.append(m)
    return in_maps


_NC_CACHE = {}


def run_hw(trace=False, **inputs):
    """Build+compile+run on 8 NeuronCores; returns (out [8,1024,768], results)."""
    if "nc" not in _NC_CACHE:
        _NC_CACHE["nc"] = _build()
    nc = _NC_CACHE["nc"]
    in_maps = _prep_inputs(**inputs)
    res = run_bass_kernel_spmd(nc, in_maps, list(range(N_CORES)), trace=trace)
    out = np.stack([res.results[b]["out"] for b in range(N_CORES)])
    return out, res


def kernel(**inputs):
    out, _ = run_hw(trace=False, **inputs)
    return out


# revision 6
# speedup vs baseline: 3.1838x; 1.0213x over previous
"""Causal self-attention (12 heads, T=1024, C=768, prefix P=4) on 8 TRN2 cores.

Sharding: data-parallel over batch B=8 -> one batch element per NeuronCore.
No collectives. Weights are replicated to every core.

Per-core kernel (all fp32):
  qkv projection split by destination layout:
    qT,kT  [128, T] per head-pair (transposed layout) = w_attn_slice.T @ xT
    v      [T, 12*65] natural layout (65th col per head = 1.0 for the
           softmax denominator), = xT_slice.T @ w_v
  prefix k/v (4 positions) are appended at the END of the kv axis, so the
  causal structure is block lower-triangular in (kv-chunk, t-chunk) space:
    scores^T tile (r, window c): psum = kT_slice.T @ qT_window  [128kv, 512t]
    e = exp(0.125 * psum)  (no max subtraction: |scores| ~ 2)
    diagonal band tiles multiplied by a 128x128 triangular 0/1 mask;
    fully-masked columns are never computed nor read.
  AV: y[tchunk] = sum_r e^T(r).T @ v_aug(r)  -> psum [128t, 65]
    col 64 = softmax denominator; normalize via DVE reciprocal +
    per-partition tensor_scalar_mul.  Two heads share a [128,128] y tile,
    one PE transpose each -> yT pair tiles [128, T].
  out = yT.T @ w_proj + b_proj  -> [T, 768] -> DMA out.
"""

import numpy as np
from contextlib import ExitStack

import concourse.bass as bass
import concourse.mybir as mybir
import concourse.tile as tile
from concourse import bacc
from concourse.bass_utils import run_bass_kernel_spmd

F32 = mybir.dt.float32
F32R = mybir.dt.float32r
F16 = mybir.dt.float16
N_CORES = 8
T, C, H, D, PFX = 1024, 768, 12, 64, 4
NPAIR = H // 2          # 6 head pairs
KC = C // 128           # 6 contraction chunks
W = 512                 # T window for scores
NW = T // W             # 2 windows
TCH = T // 128          # 8 T chunks
EXP = mybir.ActivationFunctionType.Exp
SCALE = 1.0 / np.sqrt(D)


def _build():
    nc = bacc.Bacc("TRN2", target_bir_lowering=False, debug=False,
                   num_devices=N_CORES)
    xT_d = nc.declare_dram_parameter("xT", [C, T], F32, isOutput=False)
    wqk_d = nc.declare_dram_parameter("w_qk", [C, 2 * C], F16, isOutput=False)
    wv_d = nc.declare_dram_parameter("w_v", [C, C], F16, isOutput=False)
    wp_d = nc.declare_dram_parameter("w_p", [C, C], F32, isOutput=False)
    bqk_d = nc.declare_dram_parameter("b_qk", [128, 12], F32, isOutput=False)
    bv_d = nc.declare_dram_parameter("bv_bc", [128, C], F32, isOutput=False)
    bp_d = nc.declare_dram_parameter("bp_bc", [128, C], F32, isOutput=False)
    kTc_d = nc.declare_dram_parameter("kTc", [C, PFX], F16, isOutput=False)
    vc_d = nc.declare_dram_parameter("vc_aug", [PFX, H, 65], F16, isOutput=False)
    tri_d = nc.declare_dram_parameter("tri", [128, 128], F16, isOutput=False)
    id_d = nc.declare_dram_parameter("ident", [128, 128], F32, isOutput=False)
    out_d = nc.declare_dram_parameter("out", [T, C], F32, isOutput=True)

    with tile.TileContext(nc) as tc, ExitStack() as ctx:
        pers = ctx.enter_context(tc.tile_pool(name="pers", bufs=1))
        wqkp = ctx.enter_context(tc.tile_pool(name="wqkp", bufs=12))
        qkp = ctx.enter_context(tc.tile_pool(name="qkp", bufs=2))
        ep = ctx.enter_context(tc.tile_pool(name="ep", bufs=30))
        epp = ctx.enter_context(tc.tile_pool(name="epp", bufs=6))
        yp = ctx.enter_context(tc.tile_pool(name="yp", bufs=16))
        rp = ctx.enter_context(tc.tile_pool(name="rp", bufs=4))
        op = ctx.enter_context(tc.tile_pool(name="op", bufs=2))
        ps = ctx.enter_context(tc.tile_pool(name="ps", bufs=4, space="PSUM"))
        psp = ctx.enter_context(tc.tile_pool(name="psp", bufs=1, space="PSUM"))
        pyp = ctx.enter_context(tc.tile_pool(name="pyp", bufs=2, space="PSUM"))
        ptp = ctx.enter_context(tc.tile_pool(name="ptp", bufs=1, space="PSUM"))

        # ---- persistent loads ----
        xt = []
        for k in range(KC):
            t_ = pers.tile([128, T], F32, tag=f"xt{k}")
            nc.sync.dma_start(t_[:], xT_d[128 * k:128 * k + 128, :])
            xt.append(t_)
        wv = []
        wp = []
        for k in range(KC):
            t_ = pers.tile([128, C], F16, tag=f"wv{k}")
            nc.sync.dma_start(t_[:], wv_d[128 * k:128 * k + 128, :])
            wv.append(t_)
            t_ = pers.tile([128, C], F32R, tag=f"wp{k}")
            nc.sync.dma_start(t_[:], wp_d[128 * k:128 * k + 128, :].bitcast(F32R))
            wp.append(t_)
        bqk = pers.tile([128, 12], F32, tag="bqk")
        nc.sync.dma_start(bqk[:], bqk_d[:])
        bv = pers.tile([128, C], F32, tag="bv")
        nc.sync.dma_start(bv[:], bv_d[:])
        bp = pers.tile([128, C], F32, tag="bp")
        nc.sync.dma_start(bp[:], bp_d[:])
        tri = pers.tile([128, 128], F16, tag="tri")
        nc.sync.dma_start(tri[:], tri_d[:])
        ident = pers.tile([128, 128], F32, tag="ident")
        nc.sync.dma_start(ident[:], id_d[:])
        vc = pers.tile([PFX, H, 65], F16, tag="vc")
        nc.sync.dma_start(vc[:], vc_d[:])

        yT = [pers.tile([128, T], F32R, tag=f"yT{p}", name=f"yT{p}")
              for p in range(NPAIR)]
        xtb = []
        for k in range(KC):
            t_ = pers.tile([128, T], F16, tag=f"xtb{k}")
            nc.vector.tensor_copy(t_[:], xt[k][:])
            xtb.append(t_)

        # ---- v projection: natural layout with interleaved ones column ----
        vt = []
        for mt in range(TCH):
            v_ = pers.tile([128, H, 65], F16, tag=f"v{mt}")
            nc.vector.memset(v_[:, :, 64:65], 1.0)
            for n0, nsz in ((0, 512), (512, 256)):
                pv = ps.tile([128, 512], F32, tag="ps")
                for k in range(KC):
                    nc.tensor.matmul(pv[:, :nsz],
                                     xtb[k][:, 128 * mt:128 * mt + 128],
                                     wv[k][:, n0:n0 + nsz],
                                     start=(k == 0), stop=(k == KC - 1))
                h0, hn = n0 // 64, nsz // 64
                nc.vector.tensor_add(
                    v_[:, h0:h0 + hn, 0:64],
                    pv[:, :nsz].rearrange("a (h d) -> a h d", d=64),
                    bv[:, n0:n0 + nsz].rearrange("a (h d) -> a h d", d=64))
            vt.append(v_)

        # ---- per head-pair attention ----
        prev_ysb = []
        for p in range(NPAIR):
            # weights for this pair: [128, 2, 128] per k-chunk (q col, k col)
            wq = []
            for k in range(KC):
                t_ = wqkp.tile([128, 2, 128], F16, tag="wqk")
                src = wqk_d[128 * k:128 * k + 128, :].rearrange(
                    "a (s b) -> a s b", s=2)[:, :, 128 * p:128 * p + 128]
                nc.sync.dma_start(t_[:], src)
                wq.append(t_)

            qT = qkp.tile([128, T], F16, tag="qT")
            kT = qkp.tile([128, T + PFX], F16, tag="kT")
            for half, dst in ((0, qT), (1, kT)):
                for w in range(NW):
                    pq = ps.tile([128, 512], F32, tag="ps")
                    for k in range(KC):
                        nc.tensor.matmul(pq[:], wq[k][:, half, :],
                                         xtb[k][:, W * w:W * w + W],
                                         start=(k == 0), stop=(k == KC - 1))
                    nc.vector.tensor_scalar_add(
                        dst[:, W * w:W * w + W], pq[:],
                        bqk[:, 6 * half + p:6 * half + p + 1])
            nc.sync.dma_start(kT[:, T:T + PFX], kTc_d[128 * p:128 * p + 128, :])

            # deferred transposes of the previous pair (keeps PE fed)
            while prev_ysb:
                p_, ysb_ = prev_ysb.pop(0)
                for m in range(TCH):
                    pt = ptp.tile([128, 128], F32, tag="pt", name=f"pt{p_}_{m}")
                    nc.tensor.transpose(pt[:], ysb_[m][:], ident[:])
                    nc.vector.tensor_copy(yT[p_][:, 128 * m:128 * m + 128], pt[:])

            ysb = [yp.tile([128, 128], F32, tag="ysb", name=f"ysb{p}_{i}")
                   for i in range(TCH)]

            et = {}
            etp = {}
            for s in range(2):
                h = 2 * p + s
                qs = qT[64 * s:64 * s + 64, :]
                ks = kT[64 * s:64 * s + 64, :]
                # scores + exp (+ causal band mask) per window
                for c in range(NW):
                    for r in range(4 * c + 4):
                        e_ = ep.tile([128, W], F16, tag="et",
                                     name=f"et{p}_{s}_{c}_{r}")
                        pss = ps.tile([128, 512], F32, tag="ps",
                                      name=f"pss{p}_{s}_{c}_{r}")
                        if r >= 4 * c:  # diagonal band tile
                            j0 = 128 * r - W * c
                            nc.tensor.matmul(pss[:, j0:W], ks[:, 128 * r:128 * r + 128],
                                             qs[:, W * c + j0:W * (c + 1)],
                                             start=True, stop=True)
                            nc.scalar.activation(e_[:, j0:W], pss[:, j0:W], EXP,
                                                 scale=float(SCALE))
                            nc.vector.tensor_mul(e_[:, j0:j0 + 128],
                                                 e_[:, j0:j0 + 128], tri[:])
                        else:
                            nc.tensor.matmul(pss[:], ks[:, 128 * r:128 * r + 128],
                                             qs[:, W * c:W * (c + 1)],
                                             start=True, stop=True)
                            nc.scalar.activation(e_[:], pss[:], EXP,
                                                 scale=float(SCALE))
                        et[(s, c, r)] = e_
                    pp = psp.tile([PFX, 512], F32, tag="psp",
                                  name=f"pp{p}_{s}_{c}")
                    nc.tensor.matmul(pp[:], ks[:, T:T + PFX],
                                     qs[:, W * c:W * (c + 1)], start=True, stop=True)
                    ep_ = epp.tile([PFX, W], F16, tag="etp",
                                   name=f"etp{p}_{s}_{c}")
                    nc.scalar.activation(ep_[:], pp[:], EXP, scale=float(SCALE))
                    etp[(s, c)] = ep_
            for s in range(2):
                h = 2 * p + s
                # AV + normalize
                for c in range(NW):
                    for mloc in range(4):
                        m = 4 * c + mloc
                        py = pyp.tile([128, 65], F32, tag="py",
                                      name=f"py{p}_{s}_{m}")
                        for r in range(m + 1):
                            nc.tensor.matmul(py[:],
                                             et[(s, c, r)][:, 128 * mloc:128 * mloc + 128],
                                             vt[r][:, h, :],
                                             start=(r == 0), stop=False)
                        nc.tensor.matmul(py[:],
                                         etp[(s, c)][:, 128 * mloc:128 * mloc + 128],
                                         vc[:, h, :], start=False, stop=True)
                        rc = rp.tile([128, 1], F32, tag="rc", name=f"rc{p}_{s}_{m}")
                        nc.vector.reciprocal(rc[:], py[:, 64:65])
                        nc.vector.tensor_scalar_mul(
                            ysb[m][:, 64 * s:64 * s + 64], py[:, 0:64], rc[:])

            prev_ysb.append((p, ysb))

        # drain remaining transposes
        for p_, ysb_ in prev_ysb:
            for m in range(TCH):
                pt = ptp.tile([128, 128], F32, tag="pt", name=f"pt{p_}_{m}")
                nc.tensor.transpose(pt[:], ysb_[m][:], ident[:])
                nc.vector.tensor_copy(yT[p_][:, 128 * m:128 * m + 128], pt[:])

        # ---- output projection ----
        for mt in range(TCH):
            osb = op.tile([128, C], F32, tag="osb")
            for n0, nsz in ((0, 512), (512, 256)):
                po = ps.tile([128, 512], F32, tag="ps")
                for kp in range(NPAIR):
                    nc.tensor.matmul(po[:, :nsz],
                                     yT[kp][:, 128 * mt:128 * mt + 128],
                                     wp[kp][:, n0:n0 + nsz],
                                     start=(kp == 0), stop=(kp == NPAIR - 1))
                nc.vector.tensor_add(osb[:, n0:n0 + nsz], po[:, :nsz],
                                     bp[:, n0:n0 + nsz])
            nc.sync.dma_start(out_d[128 * mt:128 * mt + 128, :], osb[:])

    nc.finalize()
    return nc


def _prep_inputs(x, kv_cvec, w_attn, b_attn, w_proj, b_proj):
    x = np.asarray(x, np.float32)
    kv_cvec = np.asarray(kv_cvec, np.float32)
    w_attn = np.asarray(w_attn, np.float32)
    b_attn = np.asarray(b_attn, np.float32)
    w_proj = np.asarray(w_proj, np.float32)
    b_proj = np.asarray(b_proj, np.float32)

    shared = {
        "w_qk": np.ascontiguousarray(w_attn[:, :2 * C]).astype(np.float16),
        "w_v": np.ascontiguousarray(w_attn[:, 2 * C:]).astype(np.float16),
        "w_p": np.ascontiguousarray(w_proj),
        "b_qk": np.ascontiguousarray(b_attn[:2 * C].reshape(12, 128).T),
        "bv_bc": np.ascontiguousarray(
            np.broadcast_to(b_attn[2 * C:], (128, C))),
        "bp_bc": np.ascontiguousarray(np.broadcast_to(b_proj, (128, C))),
        "tri": (np.arange(128)[:, None] <= np.arange(128)[None, :]
                ).astype(np.float16),
        "ident": np.eye(128, dtype=np.float32),
    }
    in_maps = []
    for b in range(N_CORES):
        vc_aug = np.zeros((PFX, H, 65), np.float32)
        vc_aug[:, :, :64] = kv_cvec[b][:, C:].reshape(PFX, H, D)
        vc_aug[:, :, 64] = 1.0
        m = dict(shared)
        m["xT"] = np.ascontiguousarray(x[b].T)
        m["kTc"] = np.ascontiguousarray(kv_cvec[b][:, :C].T
                                        ).astype(np.float16)
        m["vc_aug"] = vc_aug.astype(np.float16)
        in_maps.append(m)
    return in_maps


_NC_CACHE = {}


def run_hw(trace=False, **inputs):
    """Build+compile+run on 8 NeuronCores; returns (out [8,1024,768], results)."""
    if "nc" not in _NC_CACHE:
        _NC_CACHE["nc"] = _build()
    nc = _NC_CACHE["nc"]
    in_maps = _prep_inputs(**inputs)
    res = run_bass_kernel_spmd(nc, in_maps, list(range(N_CORES)), trace=trace)
    out = np.stack([res.results[b]["out"] for b in range(N_CORES)])
    return out, res


def kernel(**inputs):
    out, _ = run_hw(trace=False, **inputs)
    return out
